# revision 9
# baseline (speedup 1.0000x reference)
"""Trainium2 Bass kernel: batched single-head self-attention.

Reference computation (per (b, l) pair, 20 independent blocks):
    X = x[b, l] viewed as [N=1024, D=256] (xf layout)
    out[b, l] = softmax(beta * X @ X.T, axis=-1) @ X

Device algorithm (per block):
  * Scores: S[m, n] = sum_d X^T[d, m] X^T[d, n] on the TensorEngine with
    D on partitions.  All matmul MOVING operands are bf16: the PE stream
    rate is SBUF-feed-bandwidth limited, so bf16 streams at 1 cyc/col
    (measured 259ns cadence per 512-col matmul) where fp32r takes ~2.
    bf16 scores cost ~7e-3 rel-max error on this data (vs the 2e-2
    gate): verified offline against an fp64 oracle.
  * Softmax shift: W[m, n] = exp(beta * (S[m, n] - c_n)) with
    c_n = ||x_n||^2.  The per-query shift is applied OFF the PE: the
    host replicates -c across all 128 partitions, and the VectorE adds
    it to the PSUM score tile while writing the shifted fp32 copy to
    SBUF (scalar_tensor_tensor).  ScalarE exps that straight to bf16 W.
  * Second matmul RESTRUCTURED vs the classic xfo-stationary form:
    O[n, d] = sum_m W[m, n] xfo[m, d] with the W tile slice [128, 128]
    STATIONARY and xfo[m, 0:258] = [x | 1 | 0] the moving operand.
    Every streamed column now feeds 128 output rows, and the softmax
    denominator Z_n falls out as output column 256 (the ones column) --
    the 16 separate Z matmul streams per block of the previous version
    are gone entirely.  8 q-tiles x 8 key tiles = 64 matmuls of 258
    columns per block vs 48 of 512: ~2.6us/block less PE time.
  * PSUM (8 banks): 4 score tiles + 4 O accumulators -> the O matmuls
    run in two phases (q 0..3 accumulated inside the key loop, q 4..7
    re-streamed after it; W tiles stay in SBUF anyway).
  * Software pipelining: phase-1 O matmuls for key tile a are emitted
    after the score matmuls for key tile a+2, so the PE never waits on
    the VectorE add + ScalarE exp chain (~1.7us deep).

Host pre/post (layout + O(N*D) work only; all O(N^2*D) flops on device):
  * xb   = X^T in bf16                  (score operands)
  * xf   = [X | 1 | 0] in bf16          (value operand)
  * nb   = -||x_n||^2 replicated to 128 partitions (fp32 shift tile)
  * out  = O[:, :256] / O[:, 256:257]   (normalize; already [n, d])

Sharding: 20 blocks over 8 cores as 2 full blocks + 1 half block (512
queries) per core -- exact, no padded compute.  The half blocks use a
host-side rotation of the key axis so every core runs the identical
program (softmax is invariant to key permutation when values are
permuted identically).
"""

import numpy as np
import ml_dtypes

import concourse.tile as tile
from concourse import bacc, mybir
from concourse.bass_utils import run_bass_kernel_spmd

F32 = mybir.dt.float32
BF16 = mybir.dt.bfloat16

B, L, D, H, W = 4, 5, 256, 32, 32
N = H * W            # 1024 keys per block
NBLK = B * L         # 20
NCORES = 8
NFULL = 2            # full blocks per core
NSLAB = 3            # 2 full + 1 half
DF = 272             # value operand row: [x | 1 | 0 | pad...] -- padded so
                     # bf16 rows stay 32B-aligned (272*2 = 544 = 17*32)
DO = 258             # O matmul moving width / output row: [d0..d255, Z, 0]

EXP = mybir.ActivationFunctionType.Exp
ALU = mybir.AluOpType


def build_program(beta: float):
    nc = bacc.Bacc("TRN2", target_bir_lowering=False, debug=False,
                   num_devices=NCORES)
    xb_in = nc.dram_tensor("xb_in", [NSLAB, 128, 2, N], BF16,
                           kind="ExternalInput")
    xf_in = nc.dram_tensor("xf_in", [NSLAB, 128, 8, DF], BF16,
                           kind="ExternalInput")
    nb_in = nc.dram_tensor("nb_in", [1, NSLAB * N], F32,
                           kind="ExternalInput")
    y_out = nc.dram_tensor("y_out", [NSLAB, 128, 8, DO], F32,
                           kind="ExternalOutput")

    with tile.TileContext(nc) as tc:
        _build(tc, nc, xb_in.ap(), xf_in.ap(), nb_in.ap(), y_out.ap(), beta)
    nc.finalize()
    return nc


def _build(tc, nc, xb_in, xf_in, nb_in, y_out, beta):
    import contextlib
    ctx = contextlib.ExitStack()
    with ctx:
        const = ctx.enter_context(tc.tile_pool(name="const", bufs=1))
        xb_pool = ctx.enter_context(tc.tile_pool(name="xb", bufs=NSLAB))
        xfo_pool = ctx.enter_context(tc.tile_pool(name="xfo", bufs=NSLAB))
        nb_pool = ctx.enter_context(tc.tile_pool(name="nb", bufs=1))
        ssh_pool = ctx.enter_context(tc.tile_pool(name="ssh", bufs=6))
        # W tiles stay live until phase 2 at the end of the block.
        w_pool = ctx.enter_context(tc.tile_pool(name="w", bufs=10))
        o_sb_pool = ctx.enter_context(tc.tile_pool(name="o_sb", bufs=2))
        # PSUM: 4 score tiles + 4 O accumulators = 8 banks.
        ps_s = ctx.enter_context(tc.tile_pool(name="ps_s", bufs=4, space="PSUM"))
        ps_o = ctx.enter_context(tc.tile_pool(name="ps_o", bufs=4, space="PSUM"))

        # Warm the PE clock (HAM) with a throwaway full-array fp32 matmul
        # that runs during the input-DMA window -- otherwise the first
        # ~4us of real matmuls run at reduced clock.
        warm_src = const.tile([128, 512], F32)
        nc.gpsimd.memset(warm_src[:], 0.0)
        warm_ps = ps_o.tile([128, 512], F32, tag="o", name="warm_ps")
        nc.tensor.matmul(warm_ps[:], warm_src[:, 0:128], warm_src[:],
                         start=True, stop=True)

        # Input DMAs.  The first matmuls gate on xb slab 0: its two
        # chunks go FIRST on the two DMA queues so they land in the
        # first HBM wave (~1.5us), with everything else queued behind.
        # The shift rows arrive as a single 12KB DRAM row and are
        # replicated across partitions on the (otherwise idle) GpSimd,
        # keeping 1.5MB of replicated fp32 out of the HBM window.
        xbs, xfos = [], []
        xb0 = xb_pool.tile([128, 2, N], BF16, tag="xb", name="xb_0")
        nc.sync.dma_start(out=xb0[:, 0], in_=xb_in[0][:, 0])
        nc.scalar.dma_start(out=xb0[:, 1], in_=xb_in[0][:, 1])
        nb_row = nb_pool.tile([1, NSLAB * N], F32, tag="nb_row")
        nc.scalar.dma_start(out=nb_row[:], in_=nb_in[:])
        xbs.append(xb0)
        xfo0 = xfo_pool.tile([128, 8, DF], BF16, tag="xfo", name="xfo_0")
        nc.sync.dma_start(out=xfo0[:], in_=xf_in[0])
        xfos.append(xfo0)
        for s in range(1, NSLAB):
            xb = xb_pool.tile([128, 2, N], BF16, tag="xb", name=f"xb_{s}")
            nc.sync.dma_start(out=xb[:], in_=xb_in[s])
            xbs.append(xb)
            xfo = xfo_pool.tile([128, 8, DF], BF16, tag="xfo",
                                name=f"xfo_{s}")
            nc.scalar.dma_start(out=xfo[:], in_=xf_in[s])
            xfos.append(xfo)
        # replicate -c to all 128 partitions, 512 queries at a time in
        # consumption order
        nb_all = nb_pool.tile([128, NSLAB * N], F32, tag="nb")
        for s in range(NSLAB):
            n_h = 2 if s < NFULL else 1
            for h in range(n_h):
                cs = slice(s * N + h * 512, s * N + (h + 1) * 512)
                nc.gpsimd.partition_broadcast(nb_all[:, cs], nb_row[:, cs])

        for s in range(NSLAB):
            n_q = N if s < NFULL else N // 2
            n_h = n_q // 512    # 512-column query groups for the scores
            n_t = n_q // 128    # 128-query tiles for the O matmuls
            xb, xfo = xbs[s], xfos[s]

            wt_tiles = [w_pool.tile([128, N], BF16, tag="w",
                                    name=f"w_{s}_{a}") for a in range(8)]

            def emit_scores(a):
                asl = slice(a * 128, (a + 1) * 128)
                for h in range(n_h):
                    hs = slice(h * 512, (h + 1) * 512)
                    sp = ps_s.tile([128, 512], F32, tag="sps",
                                   name=f"sps_{s}_{a}_{h}")
                    for c in range(2):
                        nc.tensor.matmul(sp[:], xb[:, c, asl],
                                         xb[:, c, hs],
                                         start=(c == 0), stop=(c == 1))
                    # shift on VectorE: s_sh = S + (-c_n)  (PSUM->SBUF)
                    ssh = ssh_pool.tile([128, 512], F32, tag="ssh",
                                        name=f"ssh_{s}_{a}_{h}")
                    nc.vector.scalar_tensor_tensor(
                        ssh[:], sp[:], 1.0,
                        nb_all[:, s * N + h * 512: s * N + (h + 1) * 512],
                        ALU.mult, ALU.add)
                    # W = exp(beta * s_sh) -> bf16, on ScalarE
                    nc.scalar.activation(wt_tiles[a][:, hs], ssh[:], EXP,
                                         scale=float(beta))

            o_tiles = {}

            def emit_o(a, q0, q1, phase):
                # O[q] += W[a][:, q].T @ xfo[a]  (W slice stationary; the
                # 258-wide moving operand covers [x | 1 | 0], so column
                # 256 of the output accumulates Z)
                for q in range(q0, q1):
                    if a == 0:
                        o_tiles[q] = ps_o.tile([128, DO], F32, tag="o",
                                               name=f"o_{s}_{phase}_{q}")
                    qs = slice(q * 128, (q + 1) * 128)
                    nc.tensor.matmul(o_tiles[q][:], wt_tiles[a][:, qs],
                                     xfo[:, a, 0:DO],
                                     start=(a == 0), stop=(a == 7))

            def evac_dma(q0, q1):
                # evacuate PSUM->SBUF in pairs split across DVE and ACT,
                # with the output DMA chasing each pair so the tail
                # overlaps copy and DMA
                for p0 in range(q0, q1, 2):
                    nc.vector.tensor_copy(o_sb[:, p0, :], o_tiles[p0][:])
                    nc.scalar.copy(o_sb[:, p0 + 1, :], o_tiles[p0 + 1][:])
                    nc.sync.dma_start(out=y_out[s][:, p0:p0 + 2, :],
                                      in_=o_sb[:, p0:p0 + 2, :])

            o_sb = o_sb_pool.tile([128, 8, DO], F32, tag="o_sb")
            np1 = min(n_t, 4)   # phase-1 q tiles
            # software pipeline: phase-1 O matmuls trail the scores by 3
            # key tiles so the PE never waits on the VectorE/ScalarE
            # chain that produces W.
            for a in range(8):
                emit_scores(a)
                if a >= 3:
                    emit_o(a - 3, 0, np1, 1)
            for a in (5, 6, 7):
                emit_o(a, 0, np1, 1)
            evac_dma(0, np1)
            if n_t > 4:
                for a in range(8):
                    emit_o(a, 4, 8, 2)
                evac_dma(4, 8)


_PROG_CACHE = {}


def _get_program(beta: float):
    if beta not in _PROG_CACHE:
        _PROG_CACHE[beta] = build_program(beta)
    return _PROG_CACHE[beta]


def make_in_maps(x: np.ndarray):
    """Shard the full input [B, L, D, H, W] into 8 per-core input maps."""
    xt_all = np.ascontiguousarray(x.reshape(NBLK, D, N))
    in_maps = []
    for c in range(NCORES):
        half_blk = NFULL * NCORES + c // 2
        half = xt_all[half_blk]
        if c % 2 == 1:
            # rotate keys so this core's queries are columns 0..511
            half = np.concatenate([half[:, N // 2:], half[:, :N // 2]], axis=1)
        slabs = np.stack([xt_all[NFULL * c], xt_all[NFULL * c + 1], half])
        xf = np.zeros((NSLAB, N, DF), np.float32)
        xf[:, :, :D] = slabs.transpose(0, 2, 1)
        xf[:, :, D] = 1.0
        negc = -np.einsum('sdn,sdn->sn', slabs, slabs)
        # pack into device layout: xb [128, 2, N], xf [128, 8, DF]
        xb_p = slabs.reshape(NSLAB, 2, 128, N).transpose(0, 2, 1, 3)
        xf_p = xf.reshape(NSLAB, 8, 128, DF).transpose(0, 2, 1, 3)
        in_maps.append({
            "xb_in": np.ascontiguousarray(xb_p.astype(ml_dtypes.bfloat16)),
            "xf_in": np.ascontiguousarray(xf_p.astype(ml_dtypes.bfloat16)),
            "nb_in": np.ascontiguousarray(negc.reshape(1, NSLAB * N)),
        })
    return in_maps


def assemble_output(results):
    """Normalize and gather per-core outputs into [B, L, N, D]."""
    out = np.empty((NBLK, N, D), np.float32)
    for c in range(NCORES):
        # y [NSLAB, 128, 8, DO]: [q-within-tile, q-tile, feature]
        y = results[c]["y_out"].transpose(0, 2, 1, 3).reshape(NSLAB, N, DO)
        for s, blk, lo, n_q in ((0, NFULL * c, 0, N),
                                (1, NFULL * c + 1, 0, N),
                                (2, NFULL * NCORES + c // 2,
                                 (c % 2) * (N // 2), N // 2)):
            o = y[s, :n_q]
            out[blk, lo:lo + n_q] = o[:, :D] / o[:, D:D + 1]
    return out.reshape(B, L, N, D)


def kernel(x, beta, _trace=False, _fast=True):
    x = np.asarray(x, dtype=np.float32)
    assert x.shape == (B, L, D, H, W), x.shape
    beta_f = float(np.asarray(beta))
    prog = _get_program(beta_f)
    in_maps = make_in_maps(x)
    res = run_bass_kernel_spmd(prog, in_maps, core_ids=list(range(NCORES)),
                               trace=_trace)
    out = assemble_output(res.results)
    if _trace:
        return out, res
    return out
